# revision 17
# baseline (speedup 1.0000x reference)
"""CorrCosine TRN2 kernel.

out[b, i, j, h, w] = <cur[b,:,i,j]/||cur[b,:,i,j]||, ref[b,:,h,w]/||ref[b,:,h,w]||>

Data-parallel over batch B=8 across the 8 NeuronCores; per core one
[4096 x 256] @ [256 x 4096] GEMM plus the two L2 normalizations.

The baseline (fp32 in/out, f32r GEMM) was output-DMA-bound: the 64 MiB
fp32 result alone is ~190us of HBM write per core. This version runs the
whole pipeline in fp16 (inputs cast on host, GEMM operands fp16 with
fp32 PSUM accumulation, output stored fp16 and upcast on host), halving
both DMA traffic and PE power. Norms are computed on-chip in fp32 via an
all-ones stationary matmul (which leaves the result broadcast across all
128 partitions), and BOTH operands are pre-scaled by 1/norm on the
otherwise-idle GpSimd engine, so PSUM evacuation is a plain copy.
"""

import numpy as np

from concourse import bacc, mybir
from concourse import tile
from concourse.bass_utils import run_bass_kernel_spmd

B, C, H, W = 8, 256, 64, 64
HW = H * W            # 4096
P = 128               # partitions
KT = C // P           # 2 k-tiles
FD = 512              # psum bank free dim (fp32)
NCH = HW // FD        # 8 norm chunks
MT = HW // P          # 32 m-tiles
IBW = 2048            # input DMA width (512 KiB fp16 chunks)

f16 = mybir.dt.float16
f32 = mybir.dt.float32
f32r = mybir.dt.float32r
AF = mybir.ActivationFunctionType

_cached_nc = None


def _build():
    nc = bacc.Bacc("TRN2", target_bir_lowering=False, debug=False)
    cur_d = nc.dram_tensor("cur", [C, HW], f16, kind="ExternalInput")
    ref_d = nc.dram_tensor("ref", [C, HW], f16, kind="ExternalInput")
    out_d = nc.dram_tensor("out", [HW, HW], f16, kind="ExternalOutput")

    with tile.TileContext(nc) as tc:
        with (
            tc.tile_pool(name="opnd", bufs=1) as opnd,
            tc.tile_pool(name="cst", bufs=1) as cstp,
            tc.tile_pool(name="ps", bufs=8, space="PSUM") as psp,
        ):
            ones_f = cstp.tile([P, P], f32, tag="ones_f", name="ones_f")
            nc.gpsimd.memset(ones_f[:], 1.0)
            ones = cstp.tile([P, P], f16, tag="ones", name="ones")
            nc.vector.tensor_copy(ones[:], ones_f[:])

            raw = {}
            scl = {}
            for t in ("ref", "cur"):
                for k in range(KT):
                    raw[t, k] = opnd.tile(
                        [P, HW], f16, tag=f"raw_{t}{k}", name=f"raw_{t}{k}"
                    )
                    scl[t, k] = opnd.tile(
                        [P, HW], f16, tag=f"scl_{t}{k}", name=f"scl_{t}{k}"
                    )

            # inputs on both HWDGE rings: h0 (first 2048 cols) finely chopped
            # on the sync ring so normalization starts ASAP; h1 on the
            # scalar ring in parallel.  ref before cur on each ring.
            QBW = 1024
            for t in ("ref", "cur"):
                src = ref_d if t == "ref" else cur_d
                for i in range(IBW // QBW):
                    for k in range(KT):
                        nc.sync.dma_start(
                            raw[t, k][:, i * QBW:(i + 1) * QBW],
                            src[k * P:(k + 1) * P, i * QBW:(i + 1) * QBW],
                        )
            for t in ("ref", "cur"):
                src = ref_d if t == "ref" else cur_d
                for k in range(KT):
                    nc.scalar.dma_start(
                        raw[t, k][:, IBW:HW],
                        src[k * P:(k + 1) * P, IBW:HW],
                    )

            with (
                tc.tile_pool(name="sq", bufs=8) as sqp,
                tc.tile_pool(name="nrm", bufs=5) as nrmp,
            ):
                # normalization in two phases so the in-order GpSimd queue
                # never has a recip-dependent mul in front of a square:
                # norm_sq emits the squares (k0 on ACT, k1 on GpSimd),
                # norm_fin the ones-matmul / sqrt / recip / scale-muls
                # (k0 on DVE, k1 on GpSimd).  fp16 squares keep the
                # ones-matmul on the fast PE path.
                sqs = {}

                def norm_sq(t, ch):
                    sl = slice(ch * FD, (ch + 1) * FD)
                    sq0 = sqp.tile([P, FD], f16, tag="sq", name=f"sq0_{t}{ch}")
                    sq1 = sqp.tile([P, FD], f16, tag="sq", name=f"sq1_{t}{ch}")
                    nc.scalar.activation(sq0[:], raw[t, 0][:, sl], AF.Square)
                    nc.gpsimd.tensor_mul(sq1[:], raw[t, 1][:, sl], raw[t, 1][:, sl])
                    sqs[t, ch] = (sq0, sq1)

                def norm_fin(t, ch):
                    sl = slice(ch * FD, (ch + 1) * FD)
                    sq0, sq1 = sqs.pop((t, ch))
                    ss = psp.tile([P, FD], f32, tag="ss", name=f"ss_{t}{ch}", bufs=2)
                    nc.tensor.matmul(ss[:], ones[:], sq0[:], start=True, stop=False)
                    nc.tensor.matmul(ss[:], ones[:], sq1[:], start=False, stop=True)
                    nrm = nrmp.tile([P, FD], f32, tag="nrm", name=f"nrm_{t}{ch}")
                    nc.scalar.activation(nrm[:], ss[:], AF.Sqrt)
                    inv = nrmp.tile([P, FD], f32, tag="inv", name=f"inv_{t}{ch}")
                    nc.vector.reciprocal_approx_fast(inv[:], nrm[:])
                    nc.vector.tensor_mul(scl[t, 0][:, sl], raw[t, 0][:, sl], inv[:])
                    nc.gpsimd.tensor_mul(scl[t, 1][:, sl], raw[t, 1][:, sl], inv[:])

                def norm_scale(t, ch):
                    norm_sq(t, ch)
                    norm_fin(t, ch)

                # --- main GEMM: out[m*128:, :] = scl_cur[:, m].T @ scl_ref ---
                # emitted per half-m-tile (2 psum tiles, then a 512 KiB DMA
                # on the otherwise-idle sync ring) so PSUM drain never
                # couples the ACT queue to DVE completions, and so m0's
                # first half can run before ref h1 is normalized.
                with tc.tile_pool(name="outp", bufs=6) as outp:
                    obs = {}

                    def gemm_half(m, half):
                        msl = slice(m * P, (m + 1) * P)
                        if half == 0:
                            obs[m] = outp.tile([P, HW], f16, tag="ob", name=f"ob{m}")
                        ob = obs[m]
                        for q in (2 * half, 2 * half + 1):
                            pt = psp.tile(
                                [P, 2 * FD], f32, tag="pt", name=f"pt{m}_{q}", bufs=3
                            )
                            for sub in range(2):
                                n = q * 2 + sub
                                nsl = slice(n * FD, (n + 1) * FD)
                                psl = slice(sub * FD, (sub + 1) * FD)
                                nc.tensor.matmul(
                                    pt[:, psl], scl["cur", 0][:, msl],
                                    scl["ref", 0][:, nsl],
                                    start=True, stop=False,
                                )
                                nc.tensor.matmul(
                                    pt[:, psl], scl["cur", 1][:, msl],
                                    scl["ref", 1][:, nsl],
                                    start=False, stop=True,
                                )
                            osl = slice(q * 2 * FD, (q + 1) * 2 * FD)
                            # evac psum -> fp16 staging; ~44/56 ACT/DVE split
                            # (ACT also runs the squares/sqrt of the norms)
                            if q == 0 or (q == 2 and m % 4 != 3):
                                nc.scalar.activation(ob[:, osl], pt[:], AF.Copy)
                            else:
                                nc.vector.tensor_copy(ob[:, osl], pt[:])
                        hsl = slice(half * (HW // 2), (half + 1) * (HW // 2))
                        nc.sync.dma_start(out_d[msl, hsl], ob[:, hsl])

                    # head schedule: ref chunks 0-3 (h0) and cur chunk 0
                    # unblock the h0 halves of m0-m3, which keep the PE fed
                    # while ref h1 / later cur chunks are normalized.
                    for ch in range(NCH // 2):
                        norm_sq("ref", ch)
                    norm_sq("cur", 0)
                    for ch in range(NCH // 2):
                        norm_fin("ref", ch)
                    norm_fin("cur", 0)
                    for m in range(4):
                        gemm_half(m, 0)
                    for ch in range(NCH // 2, NCH):
                        norm_sq("ref", ch)
                    for ch in range(NCH // 2, NCH):
                        norm_fin("ref", ch)
                    norm_scale("cur", 1)
                    for m in range(4):
                        gemm_half(m, 1)
                    norm_scale("cur", 2)

                    mpc = MT // NCH  # m-tiles per cur chunk (4)
                    for m in range(4, MT):
                        if m % mpc == 0 and m // mpc + 2 <= NCH - 1:
                            norm_scale("cur", m // mpc + 2)
                        gemm_half(m, 0)
                        gemm_half(m, 1)

    nc.compile()
    return nc


def _get_nc():
    global _cached_nc
    if _cached_nc is None:
        _cached_nc = _build()
    return _cached_nc


def _run(cur, ref, trace=False, **kw):
    """cur/ref: [B, C, HW] float. Returns (out [B, HW, HW] f32, results)."""
    nc = _get_nc()
    cur = np.ascontiguousarray(np.asarray(cur).astype(np.float16))
    ref = np.ascontiguousarray(np.asarray(ref).astype(np.float16))
    in_maps = [{"cur": cur[b], "ref": ref[b]} for b in range(B)]
    res = run_bass_kernel_spmd(nc, in_maps, list(range(B)), trace=trace, **kw)
    out = np.stack([res.results[b]["out"] for b in range(B)]).astype(np.float32)
    return out, res


def kernel(ref_features, cur_features):
    ref = np.asarray(ref_features, np.float32).reshape(B, C, HW)
    cur = np.asarray(cur_features, np.float32).reshape(B, C, HW)
    out, _ = _run(cur, ref)
    return out.reshape(B, H, W, H, W)
